# revision 4
# baseline (speedup 1.0000x reference)
"""Trainium2 Bass kernel: masked attention with softmax over the query axis (dim 1).

Reference computation (per batch b):
    q = x_q @ Wq.T + bq            [Sq, dk]
    k = x_k @ Wk.T + bk            [Sk, dk]
    v = x_v @ Wv.T + bv            [Sk, dk]
    score = q @ k.T / sqrt(dk)     [Sq, Sk]
    score += -1e9 where mask == 0
    attn = softmax(score, axis=Sq) (softmax over the QUERY axis, i.e. per key column)
    y = attn @ v                   [Sq, dk]

Sharding: 8 cores = 4 batches x 2 Sk-halves. The softmax axis (Sq) stays whole on
every core, so each core's softmax is fully local; each core produces a partial
y (sum over its Sk half) and the host adds the two halves per batch.

Per-core layout (everything pre-transposed on the host so all matmuls contract
over the partition axis with zero on-chip transposes):
    scoreT[s, q] = kT.T @ qT  with qT,kT [dk, *] (dk on partitions, K=64)
    softmax over q  == free-axis reduction on the [s_part, q_free] tiles
    exp -> (*mask, sum) fused in one DVE scalar_tensor_tensor with accum_out
    y^T [dk, q] = sum_si v_scaled[si].T @ attn[si]  where v_scaled = v / denom
Numerics: fp16 storage for x/W/mask/attn (validated: resid_var ~4e-7 vs f64),
fp32 PSUM accumulation everywhere, masking via multiply (exp(s)*m) which is
exactly equivalent to the -1e9 additive mask for this data distribution.
"""

import numpy as np

B, SQ, SK, D_MODEL, D_K = 4, 4096, 4096, 1024, 64
N_CORES = 8
SK_SHARD = SK // (N_CORES // B)  # 2048
NEG = -1000000000.0


def emit_kernel(tc, aps, sq, sk, d, dk):
    """Emit the per-core attention kernel into TileContext tc.

    aps: dict name -> bass.AP for the DRAM tensors.
    """
    from contextlib import ExitStack

    import concourse.bass as bass  # noqa: F401
    from concourse import mybir

    nc = tc.nc
    f16 = mybir.dt.float16
    f32 = mybir.dt.float32
    AF = mybir.ActivationFunctionType
    ALU = mybir.AluOpType

    n_d = d // 128            # d_model chunks
    n_si = sk // 128          # key chunks (partition dim of scoreT)
    n_qj = sq // 512          # query chunks (psum free dim)

    x_qT, x_kT, x_vT = aps["x_qT"], aps["x_kT"], aps["x_vT"]
    maskT = aps["maskT"]
    wqT, wkT, wvT = aps["wqT"], aps["wkT"], aps["wvT"]
    bq8, bkc, bvb = aps["bq8"], aps["bkc"], aps["bvb"]
    out = aps["out"]

    with ExitStack() as ctx:
        const = ctx.enter_context(tc.tile_pool(name="const", bufs=1))
        persist = ctx.enter_context(tc.tile_pool(name="persist", bufs=1))
        stat_p = ctx.enter_context(tc.tile_pool(name="statp", bufs=1))
        y_p = ctx.enter_context(tc.tile_pool(name="yp", bufs=1))
        mask_p = ctx.enter_context(tc.tile_pool(name="maskp", bufs=2))
        psA = ctx.enter_context(tc.tile_pool(name="psA", bufs=3, space="PSUM"))
        psY = ctx.enter_context(tc.tile_pool(name="psY", bufs=4, space="PSUM"))

        # ---------------- constants (weights, biases) ----------------
        wq_sb = const.tile([128, n_d, dk], f16, name="wq_sb")
        wk_sb = const.tile([128, n_d, dk], f16, name="wk_sb")
        wv_sb = const.tile([128, n_d, dk], f16, name="wv_sb")
        for di in range(n_d):
            nc.sync.dma_start(wq_sb[:, di, :], wqT[di * 128:(di + 1) * 128, :])
            nc.sync.dma_start(wk_sb[:, di, :], wkT[di * 128:(di + 1) * 128, :])
            nc.sync.dma_start(wv_sb[:, di, :], wvT[di * 128:(di + 1) * 128, :])
        bq8_sb = const.tile([dk, 1], f32, name="bq8_sb")
        bk_sb = const.tile([dk, 1], f32, name="bk_sb")
        bvb_sb = const.tile([128, dk], f32, name="bvb_sb")
        nc.sync.dma_start(bq8_sb[:], bq8[:])
        nc.sync.dma_start(bk_sb[:], bkc[:])
        nc.sync.dma_start(bvb_sb[:], bvb[:])

        qT = persist.tile([dk, sq], f16, name="qT")        # [64, Sq] (scaled by 1/8)
        kT = persist.tile([dk, sk], f16, name="kT")        # [64, Sk]
        v_sb = persist.tile([128, n_si, dk], f32, name="v_sb")
        vs_sb = persist.tile([128, n_si, dk], f16, name="vs_sb")
        den = stat_p.tile([128, n_si], f32, name="den")
        rec = stat_p.tile([128, n_si], f32, name="rec")
        y_sb = y_p.tile([dk, sq], f32, name="y_sb")

        # ---------------- projections ----------------
        # kT[dk, s] = sum_d WkT[d, dk].T @ x_kT[d, s]   (+bk via Identity bias)
        with tc.tile_pool(name="xk", bufs=1) as xkp:
            xk_t = [xkp.tile([128, sk], f16, name=f"xk{di}") for di in range(n_d)]
            for di in range(n_d):
                nc.sync.dma_start(xk_t[di][:], x_kT[di * 128:(di + 1) * 128, :])
            for sj in range(sk // 512):
                ps = psA.tile([dk, 512], f32, name="ps_proj", tag="ps")
                for di in range(n_d):
                    nc.tensor.matmul(
                        ps[:], wk_sb[:, di, :], xk_t[di][:, sj * 512:(sj + 1) * 512],
                        start=(di == 0), stop=(di == n_d - 1))
                nc.scalar.activation(
                    kT[:, sj * 512:(sj + 1) * 512], ps[:], AF.Identity,
                    bias=bk_sb[:, :1])

        # v[s, dk] = sum_d x_vT[d, s].T @ WvT[d, dk]    (+bv broadcast add)
        with tc.tile_pool(name="xv", bufs=1) as xvp:
            xv_t = [xvp.tile([128, sk], f16, name=f"xv{di}") for di in range(n_d)]
            for di in range(n_d):
                nc.sync.dma_start(xv_t[di][:], x_vT[di * 128:(di + 1) * 128, :])
            for si in range(n_si):
                ps = psA.tile([128, dk], f32, name="ps_v", tag="ps")
                for di in range(n_d):
                    nc.tensor.matmul(
                        ps[:], xv_t[di][:, si * 128:(si + 1) * 128], wv_sb[:, di, :],
                        start=(di == 0), stop=(di == n_d - 1))
                nc.vector.tensor_add(v_sb[:, si, :], ps[:], bvb_sb[:])

        # qT[dk, q] = sum_d WqT[d, dk].T @ x_qT[d, q], scaled by 1/8 (+bq/8)
        with tc.tile_pool(name="xq", bufs=1) as xqp:
            xq_t = [xqp.tile([128, sq], f16, name=f"xq{di}") for di in range(n_d)]
            for di in range(n_d):
                nc.sync.dma_start(xq_t[di][:], x_qT[di * 128:(di + 1) * 128, :])
            for qj in range(n_qj):
                ps = psA.tile([dk, 512], f32, name="ps_q", tag="ps")
                for di in range(n_d):
                    nc.tensor.matmul(
                        ps[:], wq_sb[:, di, :], xq_t[di][:, qj * 512:(qj + 1) * 512],
                        start=(di == 0), stop=(di == n_d - 1))
                nc.scalar.activation(
                    qT[:, qj * 512:(qj + 1) * 512], ps[:], AF.Identity,
                    bias=bq8_sb[:, :1], scale=0.125)

        # ---------------- phase 1: scores, exp, mask, denominators ----------------
        attn_p = ctx.enter_context(tc.tile_pool(name="attnp", bufs=n_si))
        attn_t = []
        for si in range(n_si):
            mask_t = mask_p.tile([128, sq], f16, name="mask_t")
            nc.sync.dma_start(mask_t[:], maskT[si * 128:(si + 1) * 128, :])
            at = attn_p.tile([128, sq], f16, name="attn_t")
            attn_t.append(at)
            for qj in range(n_qj):
                ps = psA.tile([128, 512], f32, name="ps_s", tag="ps")
                nc.tensor.matmul(
                    ps[:], kT[:, si * 128:(si + 1) * 128],
                    qT[:, qj * 512:(qj + 1) * 512], start=True, stop=True)
                nc.scalar.activation(at[:, qj * 512:(qj + 1) * 512], ps[:], AF.Exp)
            # attn = exp * mask, denom = row-sum(attn), in ONE DVE pass
            nc.vector.scalar_tensor_tensor(
                at[:], at[:], 1.0, mask_t[:],
                op0=ALU.bypass, op1=ALU.mult,
                accum_out=den[:, si:si + 1])
            nc.vector.reciprocal(rec[:, si:si + 1], den[:, si:si + 1])
            nc.vector.tensor_scalar(
                vs_sb[:, si, :], v_sb[:, si, :], rec[:, si:si + 1], None,
                op0=ALU.mult)

        # ---------------- phase 2: y^T = sum_si vs[si].T @ attn[si] ----------------
        for qg in range((n_qj + 3) // 4):
            js = [j for j in range(4) if qg * 4 + j < n_qj]
            yps = [psY.tile([dk, 512], f32, name="yps", tag="yps") for _ in js]
            for si in range(n_si):
                for j in js:
                    qj = qg * 4 + j
                    nc.tensor.matmul(
                        yps[j][:], vs_sb[:, si, :],
                        attn_t[si][:, qj * 512:(qj + 1) * 512],
                        start=(si == 0), stop=(si == n_si - 1))
            for j in js:
                qj = qg * 4 + j
                nc.scalar.activation(
                    y_sb[:, qj * 512:(qj + 1) * 512], yps[j][:], AF.Copy)
        nc.sync.dma_start(out[:], y_sb[:])


def build_nc(sq=SQ, sk=SK_SHARD, d=D_MODEL, dk=D_K):
    """Build + compile the per-core Bacc module."""
    import concourse.tile as tile
    from concourse import bacc, mybir

    f16 = mybir.dt.float16
    f32 = mybir.dt.float32

    nc = bacc.Bacc("TRN2", target_bir_lowering=False, debug=False)
    aps = {
        "x_qT": nc.dram_tensor("x_qT", [d, sq], f16, kind="ExternalInput").ap(),
        "x_kT": nc.dram_tensor("x_kT", [d, sk], f16, kind="ExternalInput").ap(),
        "x_vT": nc.dram_tensor("x_vT", [d, sk], f16, kind="ExternalInput").ap(),
        "maskT": nc.dram_tensor("maskT", [sk, sq], f16, kind="ExternalInput").ap(),
        "wqT": nc.dram_tensor("wqT", [d, dk], f16, kind="ExternalInput").ap(),
        "wkT": nc.dram_tensor("wkT", [d, dk], f16, kind="ExternalInput").ap(),
        "wvT": nc.dram_tensor("wvT", [d, dk], f16, kind="ExternalInput").ap(),
        "bq8": nc.dram_tensor("bq8", [dk, 1], f32, kind="ExternalInput").ap(),
        "bkc": nc.dram_tensor("bkc", [dk, 1], f32, kind="ExternalInput").ap(),
        "bvb": nc.dram_tensor("bvb", [128, dk], f32, kind="ExternalInput").ap(),
        "out": nc.dram_tensor("out", [dk, sq], f32, kind="ExternalOutput").ap(),
    }
    with tile.TileContext(nc) as tc:
        emit_kernel(tc, aps, sq, sk, d, dk)
    nc.compile()
    return nc


def make_in_maps(x_q, x_k, x_v, mask, Wq, bq, Wk, bk, Wv, bv, sk_shard=SK_SHARD):
    """Host-side sharding + layout prep. Returns list of per-core input dicts."""
    f16 = np.float16
    n_shards = x_k.shape[1] // sk_shard
    wq16, wk16, wv16 = (np.ascontiguousarray(W.T).astype(f16) for W in (Wq, Wk, Wv))
    bq8 = (np.asarray(bq, np.float32) / 8.0).reshape(-1, 1)
    bkc = np.asarray(bk, np.float32).reshape(-1, 1).copy()
    bvb = np.ascontiguousarray(
        np.broadcast_to(np.asarray(bv, np.float32), (128, bv.shape[0])))
    xqT = [x_q[b].T.astype(f16) for b in range(x_q.shape[0])]
    in_maps = []
    for b in range(x_q.shape[0]):
        for h in range(n_shards):
            sl = slice(h * sk_shard, (h + 1) * sk_shard)
            in_maps.append({
                "x_qT": xqT[b],
                "x_kT": x_k[b, sl, :].T.astype(f16),
                "x_vT": x_v[b, sl, :].T.astype(f16),
                "maskT": mask[b, :, sl].T.astype(f16),
                "wqT": wq16, "wkT": wk16, "wvT": wv16,
                "bq8": bq8, "bkc": bkc, "bvb": bvb,
            })
    return in_maps


_NC_CACHE = {}
# test.py can set extra run_bass_kernel_spmd kwargs here (e.g. trace=True)
RUN_KWARGS = {}


def _get_nc():
    if "nc" not in _NC_CACHE:
        _NC_CACHE["nc"] = build_nc()
    return _NC_CACHE["nc"]


def kernel(**inputs):
    from concourse.bass_utils import run_bass_kernel_spmd

    x_q = np.asarray(inputs["x_q"], np.float32)
    x_k = np.asarray(inputs["x_k"], np.float32)
    x_v = np.asarray(inputs["x_v"], np.float32)
    mask = np.asarray(inputs["mask"])
    Wq, bq = np.asarray(inputs["Wq"], np.float32), np.asarray(inputs["bq"], np.float32)
    Wk, bk = np.asarray(inputs["Wk"], np.float32), np.asarray(inputs["bk"], np.float32)
    Wv, bv = np.asarray(inputs["Wv"], np.float32), np.asarray(inputs["bv"], np.float32)

    nc = _get_nc()
    in_maps = make_in_maps(x_q, x_k, x_v, mask, Wq, bq, Wk, bk, Wv, bv)
    res = run_bass_kernel_spmd(nc, in_maps, list(range(N_CORES)), **RUN_KWARGS)
    _NC_CACHE["last_res"] = res
    n_shards = N_CORES // x_q.shape[0]
    y = np.zeros((x_q.shape[0], SQ, D_K), np.float32)
    for core in range(N_CORES):
        y[core // n_shards] += res.results[core]["out"].T
    return y


# revision 6
# speedup vs baseline: 1.1970x; 1.1970x over previous
"""Trainium2 Bass kernel: masked attention with softmax over the query axis (dim 1).

Reference computation (per batch b):
    q = x_q @ Wq.T + bq; k = x_k @ Wk.T + bk; v = x_v @ Wv.T + bv
    score = q @ k.T / sqrt(dk) + (-1e9 where mask==0)
    attn = softmax(score, axis=Sq)   # softmax over the QUERY axis
    y = attn @ v

Sharding: 8 cores = 4 batches x 2 Sk-halves. The softmax axis (Sq) stays whole on
every core so softmax is fully local; each core produces a partial y (sum over its
Sk half) and the host adds the two halves per batch.

Per-core design (everything pre-transposed on the host; all matmuls contract over
the partition axis; zero on-chip transposes):
  - scoreT[s,q] tiles [128s x 512q] = kT.T @ qT with dk=64 on partitions (K=64).
    K=64 leaves half the PE array idle, so score matmuls are ROW-PACKED pairs:
    rows 0-63 compute even q-chunks, rows 64-127 odd q-chunks concurrently
    (qT2/kT2 hold interleaved/duplicated copies in both partition halves, which
    the col-packed projections produce for free).
  - exp on ScalarE in [128,1024] PSUM chunks -> fp16 attn tiles.
  - attn*=mask (uint8) fused with the softmax denominator row-sum in ONE DVE
    scalar_tensor_tensor (accum_out) -- masking by multiply is exactly
    equivalent to the -1e9 additive mask here (exp underflows to 0).
  - y^T accumulation col-packed: psum[0:64]=q-chunk j, psum[64:128]=chunk j+nq/2.
  - v-projection and y matmuls are interleaved into the si loop with lag 2 so
    the in-order PE queue never stalls on late DMAs / DVE results.
Numerics: fp16 storage, f32 PSUM accumulation (validated resid_var ~6e-7).
"""

import numpy as np

B, SQ, SK, D_MODEL, D_K = 4, 4096, 4096, 1024, 64
N_CORES = 8
SK_SHARD = SK // (N_CORES // B)  # 2048
LAG = 2
MASK_PREFETCH = 3


def emit_kernel(tc, aps, sq, sk, d, dk):
    """Emit the per-core attention kernel into TileContext tc."""
    from contextlib import ExitStack

    from concourse import mybir

    nc = tc.nc
    f16 = mybir.dt.float16
    f32 = mybir.dt.float32
    u8 = mybir.dt.uint8
    AF = mybir.ActivationFunctionType
    ALU = mybir.AluOpType

    n_d = d // 128            # d_model chunks
    n_si = sk // 128          # key chunks (partition dim of scoreT)
    n_qj = sq // 512          # query chunks of 512
    n_qp = n_qj // 2          # query chunk pairs (col-blocks of 1024)

    x_qT, x_kT, x_vT = aps["x_qT"], aps["x_kT"], aps["x_vT"]
    maskT = aps["maskT"]
    wall = aps["wall"]          # [128, 3, n_d, dk] f16: Wq/Wk/Wv d-chunks
    bias = aps["bias"]          # [128, 2+dk] f32: bq/8 | bk | bv_bcast
    out = aps["out"]

    with ExitStack() as ctx:
        const = ctx.enter_context(tc.tile_pool(name="const", bufs=1))
        persist = ctx.enter_context(tc.tile_pool(name="persist", bufs=1))
        stat_p = ctx.enter_context(tc.tile_pool(name="statp", bufs=1))
        mask_p = ctx.enter_context(tc.tile_pool(name="maskp", bufs=MASK_PREFETCH))
        psA = ctx.enter_context(tc.tile_pool(name="psA", bufs=2, space="PSUM"))
        psY = ctx.enter_context(tc.tile_pool(name="psY", bufs=1, space="PSUM"))

        # ---------------- constants ----------------
        w_sb = const.tile([128, 3, n_d, dk], f16, name="w_sb")
        nc.sync.dma_start(w_sb[:], wall[:])
        b_sb = const.tile([128, 2 + dk], f32, name="b_sb")
        nc.sync.dma_start(b_sb[:], bias[:])
        bq8 = b_sb[:, 0:1]
        bk2 = b_sb[:, 1:2]
        bvb = b_sb[:, 2:2 + dk]

        qT2 = persist.tile([128, sq // 2], f16, name="qT2")  # top: even, bot: odd
        kT2 = persist.tile([128, sk], f16, name="kT2")       # duplicated halves
        v_sb = persist.tile([128, n_si, dk], f16, name="v_sb")
        vs_sb = persist.tile([128, n_si, dk], f16, name="vs_sb")
        den = stat_p.tile([128, n_si], f32, name="den")
        rec = stat_p.tile([128, n_si], f32, name="rec")

        # ---------------- kT projection ----------------
        # kT2[dk(x2), s]: col-packed with identical halves (rhs shared)
        with tc.tile_pool(name="xk", bufs=1) as xkp:
            xk_t = [xkp.tile([128, sk], f16, name=f"xk{di}") for di in range(n_d)]
            for di in range(n_d):
                nc.sync.dma_start(xk_t[di][:], x_kT[di * 128:(di + 1) * 128, :])
            for sj in range(sk // 512):
                ps = psA.tile([128, 512], f32, name="ps_k", tag="ps")
                for di in range(n_d):
                    w = w_sb[:, 1, di, :]
                    r = xk_t[di][:, sj * 512:(sj + 1) * 512]
                    nc.tensor.matmul(ps[0:64, :], w, r, start=(di == 0),
                                     stop=(di == n_d - 1), skip_group_check=True)
                    nc.tensor.matmul(ps[64:128, :], w, r, start=(di == 0),
                                     stop=(di == n_d - 1), skip_group_check=True)
                nc.vector.tensor_scalar(
                    kT2[:, sj * 512:(sj + 1) * 512], ps[:], 1.0, bk2,
                    op0=ALU.mult, op1=ALU.add)

        # ---------------- qT projection ----------------
        # col-packed pairs; col-block cb covers q columns [1024cb, 1024cb+1024):
        #   psum[0:64] = q-chunk 2cb, psum[64:128] = q-chunk 2cb+1
        with tc.tile_pool(name="xq", bufs=2) as xqp:
            for cb in range(n_qp):
                xq_t = xqp.tile([128, n_d, 1024], f16, name="xq_t")
                for di in range(n_d):
                    nc.sync.dma_start(
                        xq_t[:, di, :],
                        x_qT[di * 128:(di + 1) * 128, cb * 1024:(cb + 1) * 1024])
                ps = psA.tile([128, 512], f32, name="ps_q", tag="ps")
                for di in range(n_d):
                    w = w_sb[:, 0, di, :]
                    nc.tensor.matmul(ps[0:64, :], w, xq_t[:, di, 0:512],
                                     start=(di == 0), stop=(di == n_d - 1),
                                     skip_group_check=True)
                    nc.tensor.matmul(ps[64:128, :], w, xq_t[:, di, 512:1024],
                                     start=(di == 0), stop=(di == n_d - 1),
                                     skip_group_check=True)
                nc.vector.tensor_scalar(
                    qT2[:, cb * 512:(cb + 1) * 512], ps[:], 0.125, bq8,
                    op0=ALU.mult, op1=ALU.add)

        # ---------------- mask prefetch + x_v load ----------------
        mask_t = []

        def alloc_mask(si):
            mt = mask_p.tile([128, sq], u8, name="mask_t")
            nc.sync.dma_start(mt[:], maskT[si * 128:(si + 1) * 128, :])
            mask_t.append(mt)

        for s in range(min(MASK_PREFETCH, n_si)):
            alloc_mask(s)

        xvp = ctx.enter_context(tc.tile_pool(name="xv", bufs=1))
        xv_t = [xvp.tile([128, sk], f16, name=f"xv{di}") for di in range(n_d)]
        for di in range(n_d):
            nc.sync.dma_start(xv_t[di][:], x_vT[di * 128:(di + 1) * 128, :])

        # ---------------- phase 1 with interleaved v-proj and y ----------------
        attn_p = ctx.enter_context(tc.tile_pool(name="attnp", bufs=n_si))
        attn_t = []
        yps = [psY.tile([128, 512], f32, name=f"yps{j}", tag=f"yps{j}")
               for j in range(n_qp)]

        def emit_v(si):
            ps = psA.tile([128, dk], f32, name="ps_v", tag="ps")
            for di in range(n_d):
                nc.tensor.matmul(
                    ps[:], xv_t[di][:, si * 128:(si + 1) * 128],
                    w_sb[:, 2, di, :],
                    start=(di == 0), stop=(di == n_d - 1))
            nc.vector.tensor_add(v_sb[:, si, :], ps[:], bvb)
            nc.vector.tensor_scalar(
                vs_sb[:, si, :], v_sb[:, si, :], rec[:, si:si + 1], None,
                op0=ALU.mult)

        def emit_y(si):
            for j in range(n_qp):
                nc.tensor.matmul(
                    yps[j][0:64, :], vs_sb[:, si, :],
                    attn_t[si][:, j * 512:(j + 1) * 512],
                    start=(si == 0), stop=(si == n_si - 1),
                    skip_group_check=True)
                nc.tensor.matmul(
                    yps[j][64:128, :], vs_sb[:, si, :],
                    attn_t[si][:, (j + n_qp) * 512:(j + n_qp + 1) * 512],
                    start=(si == 0), stop=(si == n_si - 1),
                    skip_group_check=True)

        for si in range(n_si):
            if si + MASK_PREFETCH < n_si:
                alloc_mask(si + MASK_PREFETCH)
            at = attn_p.tile([128, sq], f16, name="attn_t")
            attn_t.append(at)
            for p in range(n_qp):
                ps = psA.tile([128, 1024], f32, name="ps_s", tag="ps")
                nc.tensor.matmul(
                    ps[:, 0:512], kT2[0:64, si * 128:(si + 1) * 128],
                    qT2[0:64, p * 512:(p + 1) * 512],
                    start=True, stop=True, skip_group_check=True)
                nc.tensor.matmul(
                    ps[:, 512:1024], kT2[64:128, si * 128:(si + 1) * 128],
                    qT2[64:128, p * 512:(p + 1) * 512],
                    start=True, stop=True, skip_group_check=True)
                nc.scalar.activation(
                    at[:, p * 1024:(p + 1) * 1024], ps[:], AF.Exp)
            nc.vector.scalar_tensor_tensor(
                at[:], at[:], 1.0, mask_t[si][:],
                op0=ALU.bypass, op1=ALU.mult,
                accum_out=den[:, si:si + 1])
            nc.vector.reciprocal(rec[:, si:si + 1], den[:, si:si + 1])
            if si >= LAG:
                emit_v(si - LAG)
                emit_y(si - LAG)
        for si in range(max(n_si - LAG, 0), n_si):
            emit_v(si)
            emit_y(si)

        # ---------------- output ----------------
        y_p = ctx.enter_context(tc.tile_pool(name="yp", bufs=1))
        y_sb = y_p.tile([128, sq // 2], f16, name="y_sb")
        for j in range(n_qp):
            nc.vector.tensor_scalar(
                y_sb[:, j * 512:(j + 1) * 512], yps[j][:], 1.0, None,
                op0=ALU.mult)
        nc.sync.dma_start(out[:], y_sb[:])


def build_nc(sq=SQ, sk=SK_SHARD, d=D_MODEL, dk=D_K):
    """Build + compile the per-core Bacc module."""
    import concourse.tile as tile
    from concourse import bacc, mybir

    f16 = mybir.dt.float16
    f32 = mybir.dt.float32
    u8 = mybir.dt.uint8
    n_d = d // 128

    nc = bacc.Bacc("TRN2", target_bir_lowering=False, debug=False)
    aps = {
        "x_qT": nc.dram_tensor("x_qT", [d, sq], f16, kind="ExternalInput").ap(),
        "x_kT": nc.dram_tensor("x_kT", [d, sk], f16, kind="ExternalInput").ap(),
        "x_vT": nc.dram_tensor("x_vT", [d, sk], f16, kind="ExternalInput").ap(),
        "maskT": nc.dram_tensor("maskT", [sk, sq], u8, kind="ExternalInput").ap(),
        "wall": nc.dram_tensor("wall", [128, 3, n_d, dk], f16,
                               kind="ExternalInput").ap(),
        "bias": nc.dram_tensor("bias", [128, 2 + dk], f32,
                               kind="ExternalInput").ap(),
        "out": nc.dram_tensor("out", [128, sq // 2], f16,
                              kind="ExternalOutput").ap(),
    }
    with tile.TileContext(nc) as tc:
        emit_kernel(tc, aps, sq, sk, d, dk)
    nc.compile()
    return nc


def make_in_maps(x_q, x_k, x_v, mask, Wq, bq, Wk, bk, Wv, bv, sk_shard=SK_SHARD):
    """Host-side sharding + layout prep. Returns list of per-core input dicts."""
    f16 = np.float16
    d, dk = Wq.shape[1], Wq.shape[0]
    n_d = d // 128
    n_shards = x_k.shape[1] // sk_shard

    wall = np.empty((128, 3, n_d, dk), f16)
    for i, W in enumerate((Wq, Wk, Wv)):
        WT = W.T.astype(f16)  # [d, dk]
        for di in range(n_d):
            wall[:, i, di, :] = WT[di * 128:(di + 1) * 128, :]
    bias = np.empty((128, 2 + dk), np.float32)
    bias[:, 0] = np.tile(np.asarray(bq, np.float32) / 8.0, 128 // dk)
    bias[:, 1] = np.tile(np.asarray(bk, np.float32), 128 // dk)
    bias[:, 2:] = np.asarray(bv, np.float32)[None, :]

    xqT = [x_q[b].T.astype(f16) for b in range(x_q.shape[0])]
    in_maps = []
    for b in range(x_q.shape[0]):
        for h in range(n_shards):
            sl = slice(h * sk_shard, (h + 1) * sk_shard)
            in_maps.append({
                "x_qT": xqT[b],
                "x_kT": x_k[b, sl, :].T.astype(f16),
                "x_vT": x_v[b, sl, :].T.astype(f16),
                "maskT": np.ascontiguousarray(mask[b, :, sl].T).astype(np.uint8),
                "wall": wall, "bias": bias,
            })
    return in_maps


def unpack_out(o, sq=SQ, dk=D_K):
    """out [128, sq/2] f16 -> yT [dk, sq] f32. Top half: q-chunks 0..nq/2-1,
    bottom half: q-chunks nq/2..nq-1."""
    yT = np.empty((dk, sq), np.float32)
    half = sq // 2
    yT[:, 0:half] = o[0:dk, :].astype(np.float32)
    yT[:, half:sq] = o[64:64 + dk, :].astype(np.float32)
    return yT


_NC_CACHE = {}
# test.py can set extra run_bass_kernel_spmd kwargs here (e.g. trace=True)
RUN_KWARGS = {}


def _get_nc():
    if "nc" not in _NC_CACHE:
        _NC_CACHE["nc"] = build_nc()
    return _NC_CACHE["nc"]


def kernel(**inputs):
    from concourse.bass_utils import run_bass_kernel_spmd

    x_q = np.asarray(inputs["x_q"], np.float32)
    x_k = np.asarray(inputs["x_k"], np.float32)
    x_v = np.asarray(inputs["x_v"], np.float32)
    mask = np.asarray(inputs["mask"])
    Wq, bq = np.asarray(inputs["Wq"], np.float32), np.asarray(inputs["bq"], np.float32)
    Wk, bk = np.asarray(inputs["Wk"], np.float32), np.asarray(inputs["bk"], np.float32)
    Wv, bv = np.asarray(inputs["Wv"], np.float32), np.asarray(inputs["bv"], np.float32)

    nc = _get_nc()
    in_maps = make_in_maps(x_q, x_k, x_v, mask, Wq, bq, Wk, bk, Wv, bv)
    res = run_bass_kernel_spmd(nc, in_maps, list(range(N_CORES)), **RUN_KWARGS)
    _NC_CACHE["last_res"] = res
    n_shards = N_CORES // x_q.shape[0]
    y = np.zeros((x_q.shape[0], SQ, D_K), np.float32)
    for core in range(N_CORES):
        y[core // n_shards] += unpack_out(res.results[core]["out"]).T
    return y
